# revision 23
# baseline (speedup 1.0000x reference)
"""GAT layer Bass kernel for Trainium2, 8-core SPMD.

Sharding: core c handles batch b = c//2 and row-half ih = c%2 (512 rows of i).
Each core streams its edge slice once (memory-bound roofline).

v3: edge slabs pre-packed on the host in the device dtype (bf16 or fp8) in
the exact SBUF layout the att_e matmul wants:
e_pack[oct][(j_lo,e)=128][(i_l=8, j_hi=128)].  One DMA per octet
(2KB/partition contiguous), no on-device casts.  Adjacency bias pre-packed
bf16 {0,-1e9}, one DMA per 16-octet block.  All small weights + zT ride in
one packed bf16 tensor (single DMA on the ACT queue).

Per octet, PSUM logits A accumulate att_1 (ones x q_sum), adj bias
(sel512 selector) and att_e (8 block-diagonal matmuls).  A Pool
scalar_tensor_tensor adds att_2(+all biases+att_g) while evacuating
PSUM->SBUF bf16 into a 4-octet group tile S.  Per group of 4 octets, DVE
computes leaky-relu as T = 0.01*S (tensor_scalar, 4x mode) and
Lr = max(S, T) (tensor_tensor, 2x mode) -- single-PSUM-read rules
respected -- then ACT runs one grouped Exp into the P block.
Engine split: SP = e-DMA, PE = matmuls, Pool = +att2g evac, DVE = lrelu,
ACT = exp.
"""
import sys
sys.path.insert(0, "/opt/trn_rl_repo")
from contextlib import ExitStack

import numpy as np

import concourse.bass as bass
import concourse.tile as tile
from concourse import mybir

F32 = mybir.dt.float32
BF16 = mybir.dt.bfloat16
FP8 = mybir.dt.float8e4
AF = mybir.ActivationFunctionType
OP = mybir.AluOpType

B, N = 4, 1024
FN, FH, FE, FG = 128, 128, 16, 128
OUT, H = 128, 8
DH = OUT // H          # 16
ZIN = FN + FH          # 256
NC = 8                 # cores
NI = N // 2            # own rows per core = 512
NJH, NJL = N // 8, 8   # j = j_hi*8 + j_lo
NBLK = NI // 128       # i-blocks per core = 4
NOCT = 128 // 8        # octets per block = 16
GE = 4                 # octets per elementwise/exp group

import os
E_DT = BF16 if os.environ.get("E_DT", "fp8") == "bf16" else FP8
DMA_ONLY = os.environ.get("K_DMA_ONLY", "0") == "1"
PATHA_MOD = int(os.environ.get("K_PATHA_MOD", "0"))
PRELU_MOD = int(os.environ.get("K_PRELU_MOD", "2"))  # og%MOD==0 -> ACT Prelu
TS_ENG = os.environ.get("K_TS_ENGINE", "dve")
K_STAGE = int(os.environ.get("K_STAGE", "6"))

# wpack column offsets (bf16, [128, WP])
WP_MW = 0
WP_SKW = WP_MW + 256
WP_A2 = WP_SKW + 256
WP_AGW = WP_A2 + 16
WP_GF = WP_AGW + 8
WP_SEL = WP_GF + 1
WP_ISEL = WP_SEL + 512
WP = WP_ISEL + 512
# smallpack (f32, [1, SP_N])
SP_MB = 0
SP_SKB = 128
SP_A1B = 256
SP_A2B = 264
SP_AEB = 272
SP_AGB = 280
SP_N = 288


def _np_dt(dt):
    return mybir.dt.np(dt)


def build_core_program(nc, n_iters=1):
    d = {}
    def inp(name, shape, dt=F32):
        d[name] = nc.dram_tensor(name, shape, dt, kind="ExternalInput").ap()
    inp("e_pack", [NI // 8, 128, 1024], E_DT)
    inp("adj_pack", [NBLK, 64, NOCT, 128], BF16)
    inp("zpack", [128, 2 * N], BF16)
    inp("wpack", [128, WP], BF16)
    inp("smallpack", [1, SP_N])
    inp("bd", [128, 64], E_DT)
    inp("a1w32", [128, 2, H])
    ret = nc.dram_tensor("ret", [NI, OUT], F32, kind="ExternalOutput").ap()

    with tile.TileContext(nc) as tc:
        with ExitStack() as ctx:
            emit(ctx, tc, d, ret, n_iters)


def emit(ctx, tc, d, ret, n_iters):
    nc = tc.nc
    P = lambda name, bufs=1: ctx.enter_context(tc.tile_pool(name=name, bufs=bufs))
    PS = lambda name, bufs=1: ctx.enter_context(
        tc.tile_pool(name=name, bufs=bufs, space="PSUM"))

    const = P("const")
    pspro = PS("ps_pro", bufs=2)

    ones_bf = const.tile([128, 128], BF16)
    nc.gpsimd.memset(ones_bf[:], 1.0)
    from concourse.masks import make_identity
    ident = const.tile([128, 128], F32)
    make_identity(nc, ident[:])
    ones_row = const.tile([1, 128], F32)
    nc.gpsimd.memset(ones_row[:], 1.0)

    zp = const.tile([128, 2 * N], BF16)
    nc.scalar.dma_start(zp[:], d["zpack"][:])
    wp = const.tile([128, WP], BF16)
    nc.scalar.dma_start(wp[:], d["wpack"][:])
    a1w32 = const.tile([128, 2, H], F32)
    nc.gpsimd.dma_start(a1w32[:], d["a1w32"][:])
    bd = const.tile([128, 64], E_DT)
    nc.gpsimd.dma_start(bd[:], d["bd"][:])
    small = const.tile([1, SP_N], F32)
    nc.gpsimd.dma_start(small[:], d["smallpack"][:])

    zT = zp[:]
    def zT_half(h_idx):
        return zT[:, h_idx * N:(h_idx + 1) * N]
    m_w2 = wp[:, WP_MW:WP_MW + 256].rearrange("p (c n) -> p c n", c=2)
    m_w = lambda ct: m_w2[:, ct, :]
    sk_w2 = wp[:, WP_SKW:WP_SKW + 256].rearrange("p (c n) -> p c n", c=2)
    sk_w = lambda ct: sk_w2[:, ct, :]
    a2_w2 = wp[:, WP_A2:WP_A2 + 16].rearrange("p (c n) -> p c n", c=2)
    a2_w = lambda ct: a2_w2[:, ct, :]
    ag_w = wp[:, WP_AGW:WP_AGW + 8]
    gf = wp[:, WP_GF:WP_GF + 1]
    sel512 = wp[0:64, WP_SEL:WP_SEL + 512]
    isel = wp[0:64, WP_ISEL:WP_ISEL + 512]
    m_b = small[:, SP_MB:SP_MB + OUT]
    sk_b = small[:, SP_SKB:SP_SKB + OUT]
    a1_b = small[:, SP_A1B:SP_A1B + H]
    a2_b = small[:, SP_A2B:SP_A2B + H]
    ae_b = small[:, SP_AEB:SP_AEB + H]
    ag_b = small[:, SP_AGB:SP_AGB + H]

    # q_sum[k, (h, i)] bf16: z (x) a1_w products, ones.T @ q_sum = att_1.
    # Chunk 0 (block 0's rows) first so the first att_1 matmul unblocks
    # early; the rest in one sweep.  4x-mode tensor_scalar (per-partition
    # AP scalar) + 2x TT add, all DVE.
    q_sum = const.tile([128, H, NI], BF16)
    qpool = P("qpool", bufs=3)
    for c0, c1 in ((0, 128), (128, NI)):
        for h in range(H):
            qt0 = qpool.tile([128, NI], BF16, name="qt0", tag="qt")
            qt1 = qpool.tile([128, NI], BF16, name="qt1", tag="qt")
            nc.vector.tensor_scalar_mul(
                qt0[:, c0:c1], zT_half(0)[:, c0:c1], a1w32[:, 0, h:h + 1])
            nc.vector.tensor_scalar_mul(
                qt1[:, c0:c1], zT_half(1)[:, c0:c1], a1w32[:, 1, h:h + 1])
            nc.vector.tensor_tensor(q_sum[:, h, c0:c1], qt0[:, c0:c1],
                                    qt1[:, c0:c1], OP.add)

    # cst[h] = (a1_b + a2_b + ae_b + ag_b)[host-folded into a2_b slot]
    #          + gf @ ag_w ; assembly on Pool so DVE stays on q_sum.
    attg_ps = pspro.tile([1, H], F32, tag="pro", name="attg_ps")
    nc.tensor.matmul(attg_ps[:], gf, ag_w, start=True, stop=True)
    cstv = const.tile([1, H], F32)
    nc.vector.scalar_tensor_tensor(cstv[:], a2_b, 1.0, attg_ps[:], OP.mult, OP.add)
    cstb_ps = pspro.tile([128, H], F32, tag="pro", name="cstb_ps")
    nc.tensor.matmul(cstb_ps[:], ones_row[:], cstv[:], start=True, stop=True)
    cstb = const.tile([128, H], F32)
    nc.vector.tensor_copy(cstb[:], cstb_ps[:])

    # att2g[j_hi, (h, j_lo)] = att_2[j, h] + cst[h]   (f32, stays in SBUF)
    att2g = const.tile([128, 64], F32)
    for jl in range(NJL):
        a2ps = pspro.tile([128, H], F32, tag="pro", name="a2ps")
        for ct in range(2):
            lhs = zT_half(ct).rearrange("p (j l) -> p j l", l=8)[:, :, jl]
            nc.tensor.matmul(a2ps[:], lhs, a2_w(ct),
                             start=(ct == 0), stop=(ct == 1))
        dst = att2g[:].rearrange("p (h j) -> p h j", h=H)[:, :, jl]
        nc.vector.scalar_tensor_tensor(dst, a2ps[:], 1.0, cstb[:],
                                       OP.mult, OP.add)
    att2g_bc = att2g[:].rearrange("p (x h j) -> p x h j", x=1, h=H).broadcast_to(
        [128, 8, H, NJL])
    att2gT_ps = pspro.tile([64, 128], F32, tag="pro", name="att2gT_ps")
    nc.tensor.transpose(att2gT_ps[:], att2g[:], ident[:])
    att2gT = const.tile([64, 128], BF16)
    nc.vector.tensor_copy(att2gT[:], att2gT_ps[:])

    # V_perm[j_hi, (h, j_lo, 17)] bf16; col 16 of each (h,j_lo) group is 1.0
    v_perm = const.tile([128, H * NJL * (DH + 1)], BF16)
    nc.gpsimd.memset(v_perm[:], 1.0)
    m_b_bc_ps = pspro.tile([128, OUT], F32, tag="pro", name="mbbc")
    nc.tensor.matmul(m_b_bc_ps[:], ones_row[:], m_b, start=True, stop=True)
    m_b_bc = const.tile([128, OUT], F32)
    nc.vector.tensor_copy(m_b_bc[:], m_b_bc_ps[:])
    for jl in range(NJL):
        vps = pspro.tile([128, OUT], F32, tag="pro", name="vps")
        for ct in range(2):
            lhs = zT_half(ct).rearrange("p (j l) -> p j l", l=8)[:, :, jl]
            nc.tensor.matmul(vps[:], lhs, m_w(ct),
                             start=(ct == 0), stop=(ct == 1))
        dst = v_perm[:].rearrange("p (h j d) -> p h j d", h=H, j=NJL)[:, :, jl, 0:DH]
        nc.vector.scalar_tensor_tensor(
            dst, vps[:].rearrange("p (h d) -> p h d", h=H), 1.0,
            m_b_bc[:].rearrange("p (h d) -> p h d", h=H), OP.mult, OP.add)
    vp4 = v_perm[:].rearrange("p (h j d) -> p h j d", h=H, j=NJL)

    # ---------------- main loop ----------------
    slabp = P("slab", bufs=10)
    adjp = P("adjp", bufs=2)
    lp = PS("logits", bufs=4)
    sp_ = P("spool", bufs=5)     # S group tiles [128, GE*512] bf16
    tp_ = P("tpool", bufs=3)     # 0.01*S scratch
    lrp = P("lrpool", bufs=3)    # lrelu group tiles
    pblk = P("pblock", bufs=2)
    psavsk = PS("ps_avsk", bufs=2)
    rp = P("rasm", bufs=2)
    outp = P("outs", bufs=2)

    for it in range(n_iters):
        adjts = {}
        p_blocks = {}

        def block_tail(jb):
            # attention @ V + normalize + skip + store for finished block jb
            p_block = p_blocks.pop(jb)
            pb4 = p_block[:].rearrange("p (i h j) -> p i h j", i=128, h=H)
            av = psavsk.tile([128, H, DH + 1], F32, tag="avsk", name="av")
            for h in range(H):
                for jl in range(NJL):
                    nc.tensor.matmul(av[:, h, :], pb4[:, :, h, jl],
                                     vp4[:, h, jl, :],
                                     start=(jl == 0), stop=(jl == 7),
                                     skip_group_check=True)
            rc = rp.tile([128, H], F32, name="rc")
            nc.vector.reciprocal(rc[:], av[:, :, DH])
            r_asm = rp.tile([128, OUT], F32, name="r_asm")
            nc.vector.scalar_tensor_tensor(
                r_asm[:].rearrange("p (h d) -> p h d", h=H),
                av[:, :, 0:DH], 1.0,
                rc[:].rearrange("p (h x) -> p h x", x=1).broadcast_to([128, H, DH]),
                OP.mult, OP.mult)
            sk = psavsk.tile([128, OUT], F32, tag="avsk", name="sk")
            for ct in range(2):
                lhs = zT_half(ct)[:, jb * 128:(jb + 1) * 128]
                nc.tensor.matmul(sk[:], lhs, sk_w(ct),
                                 start=(ct == 0), stop=(ct == 1),
                                 skip_group_check=True)

            s2 = outp.tile([128, OUT], F32, name="s2")
            nc.vector.scalar_tensor_tensor(s2[:], sk[:], 1.0, r_asm[:],
                                           OP.mult, OP.add)
            ob = outp.tile([128, OUT], F32, name="ob")
            nc.scalar.activation(ob[:], s2[:], AF.Relu)
            nc.scalar.dma_start(ret[jb * 128:(jb + 1) * 128, :], ob[:])

        if DMA_ONLY:
            for ib in range(NBLK):
                probe = rp.tile([128, NOCT], F32, name="probe")
                for oct in range(NOCT):
                    t8 = slabp.tile([128, 1024], E_DT, name="t8")
                    nc.sync.dma_start(t8[:], d["e_pack"][ib * NOCT + oct])
                    nc.vector.tensor_copy(probe[:, oct:oct + 1], t8[:, 0:1])
                nc.scalar.dma_start(ret[ib * 128:(ib + 1) * 128, 0:NOCT],
                                    probe[:])
            return
        adjts[0] = adjp.tile([64, NOCT, 128], BF16, name="adjt")
        nc.gpsimd.dma_start(adjts[0][:], d["adj_pack"][0])
        for ib in range(NBLK):
            adjt = adjts.pop(ib)
            p_block = pblk.tile([128, NOCT * 512], BF16)
            p_blocks[ib] = p_block
            if K_STAGE < 6:
                stage_probe = rp.tile([128, NOCT], F32, name="sprobe")
                nc.gpsimd.memset(stage_probe[:], 0.0)
            for og in range(NOCT // GE):
                act_prelu = PRELU_MOD > 0 and og % PRELU_MOD == 0
                S = sp_.tile([128, GE * 512], BF16, name="S")
                Lr = lrp.tile([128, GE * 512], BF16, name="Lr")
                for q in range(GE):
                    oct = og * GE + q
                    gi = ib * NOCT + oct
                    path_a = PATHA_MOD > 0 and (gi % PATHA_MOD == PATHA_MOD - 1)
                    A = lp.tile([128, 512], F32)
                    A4 = A[:].rearrange("p (i h j) -> p i h j", i=8, h=H)
                    t8 = slabp.tile([128, 1024], E_DT, name="t8")
                    nc.sync.dma_start(t8[:], d["e_pack"][gi])
                    # att_e per i (first: only needs t8+bd)
                    for il in range(8):
                        nc.tensor.matmul(A[:, il * 64:(il + 1) * 64],
                                         t8[:, il * 128:(il + 1) * 128],
                                         bd[:],
                                         start=(il == 0), stop=False,
                                         skip_group_check=True)
                    # adjacency mask bias: adjt.T @ sel512
                    nc.tensor.matmul(A[:], adjt[:, oct, :], sel512,
                                     start=False, stop=False,
                                     skip_group_check=True)
                    if path_a:
                        # att_2 via PE so ACT can evacuate with a copy
                        nc.tensor.matmul(A[:], att2gT[:], isel,
                                         start=False, stop=False,
                                         skip_group_check=True)
                    # att_1 (broadcast over j) : ones.T @ q_sum-slice
                    qsl = q_sum[:, :, gi * 8:(gi + 1) * 8].rearrange(
                        "p h i -> p i h").rearrange(
                        "p i (h x) -> p i h x", x=1).broadcast_to(
                        [128, 8, H, NJL])
                    nc.tensor.matmul(A4, ones_bf[:], qsl,
                                     start=False, stop=True,
                                     skip_group_check=True)
                    if K_STAGE <= 2:
                        nc.vector.tensor_copy(
                            stage_probe[:, oct:oct + 1], A[:, 0:1])
                        continue
                    Sp = S[:, q * 512:(q + 1) * 512]
                    if path_a:
                        # ACT evacuation (att_2 already in PSUM)
                        nc.scalar.copy(Sp, A[:])
                    else:
                        # DVE evacuation + att_2 + cst add
                        nc.vector.scalar_tensor_tensor(
                            Sp.rearrange("p (i h j) -> p i h j", i=8, h=H),
                            A4, 1.0, att2g_bc, OP.mult, OP.add)
                    if K_STAGE <= 3:
                        continue
                    if act_prelu:
                        continue
                    if q % 2 == 1:
                        # leaky relu per pair: T = 0.01*S (DVE), max (DVE)
                        pr = q // 2
                        Sh = S[:, pr * 1024:(pr + 1) * 1024]
                        Tp = tp_.tile([128, 1024], BF16, name="T", tag="T")
                        if TS_ENG == "dve":
                            nc.vector.tensor_scalar_mul(Tp[:], Sh, 0.01)
                        else:
                            nc.gpsimd.tensor_scalar_mul(Tp[:], Sh, 0.01)
                        nc.vector.tensor_tensor(
                            Lr[:, pr * 1024:(pr + 1) * 1024], Sh, Tp[:],
                            OP.max)
                if K_STAGE == 3 or K_STAGE == 4:
                    srct = S if K_STAGE == 3 else Lr
                    nc.vector.tensor_copy(
                        stage_probe[:, og * GE:og * GE + 1], srct[:, 0:1])
                if og == NOCT // GE - 1 and K_STAGE < 6:
                    nc.scalar.dma_start(
                        ret[ib * 128:(ib + 1) * 128, 0:NOCT], stage_probe[:])
                if K_STAGE == 3 or K_STAGE == 4:
                    srct = S if K_STAGE == 3 else Lr
                    nc.vector.tensor_copy(
                        stage_probe[:, og * GE:og * GE + 1], srct[:, 0:1])
                if og == 0:
                    # prefetch next block's adjacency during this block
                    if ib + 1 < NBLK:
                        adjts[ib + 1] = adjp.tile([64, NOCT, 128], BF16,
                                                  name="adjt")
                        nc.gpsimd.dma_start(adjts[ib + 1][:],
                                            d["adj_pack"][ib + 1])
                    # previous block's tail rides behind this block's head
                    if K_STAGE >= 6 and ib > 0:
                        block_tail(ib - 1)
                if K_STAGE >= 5:
                    if act_prelu:
                        nc.scalar.activation(Lr[:], S[:], AF.Prelu, alpha=0.01)
                    # grouped exp -> bf16 into P block (ACT)
                    nc.scalar.activation(
                        p_block[:, og * GE * 512:(og + 1) * GE * 512],
                        Lr[:], AF.Exp)
                    if K_STAGE == 5:
                        nc.vector.tensor_copy(
                            stage_probe[:, og * GE:og * GE + 1],
                            p_block[:, og * GE * 512:og * GE * 512 + 1])
        if K_STAGE >= 6:
            block_tail(NBLK - 1)
        else:
            for jb in range(NBLK):
                p_blocks.pop(jb, None)


def split_multi_waits(nc):
    """Walrus codegen limits sem-waits per instruction (1 on Drain, ~2 on
    others). Hoist extras onto preceding wait-only NoOps on the same engine."""
    import bass_rust
    for fn in nc.m.functions:
        for bb in fn.blocks:
            out = []
            for inst in bb.instructions:
                si = inst.sync_info
                waits = list(si.on_wait) if si is not None else []
                limit = 1
                if len(waits) > limit:
                    extra, keep = waits[:-limit], waits[-limit:]
                    for i in range(len(extra)):
                        nop = mybir.InstNoOp(
                            name=nc.get_next_instruction_name(), ins=[], outs=[])
                        nop.engine = inst.engine
                        nop.sync_info = bass_rust.SyncInfo(
                            on_wait=[extra[i]], on_update=[])
                        nc.register_instruction(nop)
                        out.append(nop)
                    inst.sync_info = bass_rust.SyncInfo(
                        on_wait=keep, on_update=list(si.on_update))
                out.append(inst)
            bb.instructions[:] = out


def shard_inputs(inputs):
    """Full inputs -> list of 8 per-core in_maps (numpy)."""
    bf16 = _np_dt(BF16)
    edt = _np_dt(E_DT)
    e = np.asarray(inputs["edge_fts"], dtype=np.float32)
    nf = np.asarray(inputs["node_fts"], dtype=np.float32)
    hd = np.asarray(inputs["hidden"], dtype=np.float32)
    gfa = np.ascontiguousarray(inputs["graph_fts"], dtype=np.float32)
    adj = np.asarray(inputs["adj_mat"])
    w = {k: np.ascontiguousarray(inputs[k], dtype=np.float32) for k in (
        "m_w", "m_b", "skip_w", "skip_b", "a1_w", "a1_b", "a2_w", "a2_b",
        "ae_w", "ae_b", "ag_w", "ag_b")}
    # static selector: sel[(il,jl), (i, h, j)] = (il==i) & (jl==j)
    SEL512 = np.zeros((64, 8, 8, 8), np.float32)
    for i2 in range(8):
        for j2 in range(8):
            SEL512[i2 * 8 + j2, i2, :, j2] = 1.0
    SEL512 = SEL512.reshape(64, 512)
    # isel[(h2,jl2), (i, h, jl)] = (h2==h) & (jl2==jl)
    ISEL = np.zeros((8, 8, 8, 8, 8), np.float32)
    for h2 in range(8):
        for j2 in range(8):
            ISEL[h2, j2, :, h2, j2] = 1.0
    ISEL = ISEL.reshape(64, 512)
    # block-diagonal ae_w: bd[(jl,e), (h,jl')] = ae_w[e,h] * (jl == jl')
    bdz = np.zeros((8, 16, 8, 8), np.float32)
    for jl in range(8):
        bdz[jl, :, :, jl] = w["ae_w"]
    BD = bdz.reshape(128, 64).astype(edt)

    def two(wname):  # [256, n] -> [128, 2*n] in (c, n) minor order
        a = w[wname]
        return a.reshape(2, 128, -1).transpose(1, 0, 2).reshape(128, -1)

    smallpack = np.zeros((1, SP_N), np.float32)
    # m_b slot carries m_b + skip_b: values bias adds (m_b+skip_b)*1 to the
    # normalized attention output since softmax coefficients sum to 1.
    smallpack[0, SP_MB:SP_MB + OUT] = w["m_b"] + w["skip_b"]
    smallpack[0, SP_SKB:SP_SKB + OUT] = w["skip_b"]
    smallpack[0, SP_A1B:SP_A1B + H] = w["a1_b"]
    smallpack[0, SP_A2B:SP_A2B + H] = (w["a1_b"] + w["a2_b"] + w["ae_b"]
                                       + w["ag_b"])
    smallpack[0, SP_AEB:SP_AEB + H] = w["ae_b"]
    smallpack[0, SP_AGB:SP_AGB + H] = w["ag_b"]

    maps = []
    for c in range(NC):
        b, ih = c // 2, c % 2
        i0 = ih * NI
        # For odd cores, rotate the j axis (and z rows) by -512 so that the
        # core's own rows always sit at z columns 0..511. The attention sum
        # over j is permutation-invariant, so rolling e/adj/z consistently
        # leaves the output unchanged.
        ej = e[b, i0:i0 + NI]
        aj = adj[b, i0:i0 + NI, :]
        nfb, hdb = nf[b], hd[b]
        if ih == 1:
            ej = np.roll(ej, -NI, axis=1)
            aj = np.roll(aj, -NI, axis=1)
            nfb = np.roll(nfb, -NI, axis=0)
            hdb = np.roll(hdb, -NI, axis=0)
        # edge slabs, device dtype, DMA-final layout: [oct, (jl,e), (il, jh)]
        ejq = np.ascontiguousarray(ej).astype(edt)
        e_pack = np.ascontiguousarray(
            ejq.reshape(64, 8, 128, 8, 16).transpose(0, 3, 4, 1, 2)
            .reshape(64, 128, 1024))
        # adjacency bias bf16 {0, -1e9}: [blk4, (il,jl)=64, oct16, jh=128]
        a = ((aj.astype(np.float32) - 1.0) * 1e9).astype(bf16)
        a = a.reshape(64, 8, 128, 8).transpose(0, 1, 3, 2)     # oct, il, jl, jh
        adj_pack = np.ascontiguousarray(
            a.reshape(4, 16, 64, 128).transpose(0, 2, 1, 3))
        zpack = np.concatenate([nfb.T, hdb.T], axis=1)
        wpack = np.zeros((128, WP), np.float32)
        wpack[:, WP_MW:WP_MW + 256] = two("m_w")
        wpack[:, WP_SKW:WP_SKW + 256] = two("skip_w")
        wpack[:, WP_A2:WP_A2 + 16] = two("a2_w")
        wpack[:, WP_AGW:WP_AGW + 8] = w["ag_w"]
        wpack[:, WP_GF:WP_GF + 1] = gfa[b].reshape(FG, 1)
        wpack[0:64, WP_SEL:WP_SEL + 512] = SEL512
        wpack[0:64, WP_ISEL:WP_ISEL + 512] = ISEL
        m = {
            "e_pack": e_pack,
            "adj_pack": adj_pack,
            "zpack": zpack.astype(bf16),
            "wpack": wpack.astype(bf16),
            "smallpack": smallpack,
            "bd": BD,
            "a1w32": np.ascontiguousarray(
                w["a1_w"].reshape(2, 128, H).transpose(1, 0, 2)),
        }
        maps.append(m)
    return maps


def build(n_iters=1):
    """One program shared by all 8 cores (inputs are pre-rotated so own
    rows always sit at z columns 0..511)."""
    nc = bass.Bass("TRN2", target_bir_lowering=False, debug=False,
                   num_devices=NC)
    build_core_program(nc, n_iters=n_iters)
    split_multi_waits(nc)
    return nc


def kernel(**inputs):
    from concourse.bass_utils import run_bass_kernel_spmd
    maps = shard_inputs(inputs)
    nc = build(n_iters=1)
    res = run_bass_kernel_spmd(nc, maps, list(range(NC))).results
    out = np.zeros((B, N, OUT), np.float32)
    for c in range(NC):
        b, ih = c // 2, c % 2
        out[b, ih * NI:(ih + 1) * NI] = res[c]["ret"]
    return out
